# revision 49
# baseline (speedup 1.0000x reference)
"""Trainium2 Bass kernel for BaseWindowAttention.

Problem (hardcoded): x [2,8,64,64,256] f32, w_qkv [256,768], w_out [256,256],
b_out [256], pos_embedding [15,15], window_size 8, heads 8, dim_head 32.

Strategy:
- Data parallel: 16 (b,l) images over 8 cores -> 2 images/core.
- Host: window-major channel-first bf16 transpose of x; fold softmax scale
  into w_q; precompute exp(bias) 2-window super-tile (off-diagonal zeros kill
  cross-window attention terms); b_out added host-side after the gather.
- Device per core, per strip of 512 tokens (8 windows):
  q/k projection ([o,t] layout), v projection ([t,o] layout, head-strided with
  an appended ones column for the softmax denominator), window-pair dots as
  4 row-group-packed [32,128]x[32,128] matmuls, ACT exp, GpSimd/DVE multiply
  by exp(bias) mask tile, AV matmul (fused denominator), reciprocal +
  broadcast normalize, PE transpose to [hc,t], out-projection, per-wp
  output DMA.
- PSUM banks: qk/v projections share a 2-buf [128,512] tag, dots use two
  2-bank tiles, av/transpose/out-proj share a 2-buf small tag (8 banks
  total).  Split x DMAs ([128,256] quarters) shorten prefetch latency.
  Copies routed via nc.any so the Tile scheduler balances ACT/DVE; the
  exp(bias) mask multiply is split 3 GpSimd / 1 DVE.

Perf notes (measured on trn2 via NTFF): baseline 294us -> this version
~197us.  PE (tensor engine) is the critical resource: LDWEIGHTS fully
overlaps MATMUL, so instruction *time* not load count dominates; HAM
p-state (1.2 vs 2.4 GHz) oscillates with PE idle gaps, so PSUM slot
availability (qkps bufs=2) and deep SBUF pools matter more than per-op
engine choice.  The largest single win after PSUM rebalancing was
splitting each exp(bias) mask-multiply into two wp-pair halves with
separate edm tiles, h0 halves emitted before h1 halves (218 -> 197us):
AV for wp 0-1 then gates on only half the serialized GpSimd mul chain.
EDM_DVE=1 beats 2 or 4 in both structures.  Attempted and rejected:
DMA-XBAR transpose (Sync engine serializes ~1.2us/transpose -> 2x
slower), per-window garbage-free dots via column tile_position packing
(two stationary tiles sharing a PE row band corrupt results: one
window's outputs zero out), fp8 projections (error budget too tight).
"""

import os
import sys
import numpy as np

sys.path.insert(0, "/opt/trn_rl_repo")
os.environ.setdefault("JAX_PLATFORMS", "")

import ml_dtypes

BF16 = ml_dtypes.bfloat16

B, L, H, W, C = 2, 8, 64, 64, 256
WS = 8
NHEADS = 8
CH = 32
N_CORES = 8
IMG = B * L                 # 16 images
IMG_PER_CORE = IMG // N_CORES
T_IMG = H * W               # 4096 tokens per image
STRIP = 512                 # tokens per strip (8 windows)
N_STRIPS = T_IMG // STRIP   # 8
NWP = STRIP // 128          # 4 window pairs per strip

_CACHE = {}


def _relative_indices(ws):
    idx = np.array([[i, j] for i in range(ws) for j in range(ws)])
    rel = idx[None, :, :] - idx[:, None, :] + ws - 1
    return rel


def _build_kernel():
    import concourse.bass as bass
    import concourse.mybir as mybir
    import concourse.tile as tile
    from concourse import bacc

    dt = mybir.dt
    nc = bacc.Bacc("TRN2", target_bir_lowering=False, debug=False)

    xT = nc.dram_tensor("xT", [IMG_PER_CORE, C, T_IMG], dt.bfloat16,
                        kind="ExternalInput").ap()
    wqk = nc.dram_tensor("wqk", [C, 512], dt.bfloat16, kind="ExternalInput").ap()
    wv = nc.dram_tensor("wv", [C, C], dt.bfloat16, kind="ExternalInput").ap()
    wout = nc.dram_tensor("wout", [C, C], dt.bfloat16, kind="ExternalInput").ap()
    ebrep = nc.dram_tensor("ebrep", [128, 2048], dt.bfloat16,
                           kind="ExternalInput").ap()
    ident = nc.dram_tensor("ident", [128, 128], dt.bfloat16,
                           kind="ExternalInput").ap()
    out = nc.dram_tensor("out", [IMG_PER_CORE, T_IMG, C], dt.bfloat16,
                         kind="ExternalOutput").ap()

    EXP = mybir.ActivationFunctionType.Exp

    with tile.TileContext(nc) as tc:
        from contextlib import ExitStack
        with ExitStack() as ctx:
            consts = ctx.enter_context(tc.tile_pool(name="consts", bufs=1))
            xp = ctx.enter_context(tc.tile_pool(name="xp", bufs=3))
            qkp = ctx.enter_context(tc.tile_pool(name="qkp", bufs=8))
            vp = ctx.enter_context(tc.tile_pool(name="vp", bufs=8))
            ep = ctx.enter_context(tc.tile_pool(name="ep", bufs=3))
            anp = ctx.enter_context(tc.tile_pool(name="anp", bufs=3))
            aotp = ctx.enter_context(tc.tile_pool(name="aotp", bufs=4))
            rdp = ctx.enter_context(tc.tile_pool(name="rdp", bufs=3))
            fop = ctx.enter_context(tc.tile_pool(name="fop", bufs=3))
            psp = ctx.enter_context(tc.tile_pool(name="psp", bufs=1, space="PSUM"))

            # ---- strip-0 inputs first (head latency), then constants.
            # Big constants are split so no single DMA engine serializes
            # a large transfer in front of the first matmuls.
            pre_x = []
            for half in range(2):
                xt = xp.tile([128, STRIP], dt.bfloat16,
                             tag=("xa" if half == 0 else "xb"), bufs=6)
                for q in range(2):
                    nc.sync.dma_start(
                        out=xt[:, q * 256:(q + 1) * 256],
                        in_=xT[0, half * 128:half * 128 + 128,
                               q * 256:(q + 1) * 256])
                pre_x.append(xt)

            wqk_sb = []
            wv_sb = []
            wout_sb = []
            for kk in range(2):
                wqk_t = consts.tile([128, 512], dt.bfloat16, tag=f"wqk{kk}")
                for q in range(2):
                    nc.sync.dma_start(
                        out=wqk_t[:, q * 256:(q + 1) * 256],
                        in_=wqk[kk * 128:(kk + 1) * 128,
                                q * 256:(q + 1) * 256])
                wqk_sb.append(wqk_t)
                wv_t = consts.tile([128, 256], dt.bfloat16, tag=f"wv{kk}")
                nc.sync.dma_start(out=wv_t, in_=wv[kk * 128:(kk + 1) * 128, :])
                wv_sb.append(wv_t)
                wout_t = consts.tile([128, 256], dt.bfloat16, tag=f"wout{kk}")
                nc.sync.dma_start(out=wout_t, in_=wout[kk * 128:(kk + 1) * 128, :])
                wout_sb.append(wout_t)
            eb_sb = consts.tile([128, 4, 512], dt.bfloat16, tag="eb")
            ebr = ebrep.rearrange("p (r c) -> p r c", r=4)
            for r in range(4):
                nc.sync.dma_start(out=eb_sb[:, r, :], in_=ebr[:, r, :])
            id_sb = consts.tile([128, 128], dt.bfloat16, tag="id")
            nc.sync.dma_start(out=id_sb, in_=ident)

            QKPS_BUFS = int(os.environ.get("QKPS_BUFS", "2"))
            SMALLPS_BUFS = int(os.environ.get("SMALLPS_BUFS", "2"))
            DPS_BUFS = int(os.environ.get("DPS_BUFS", "2"))
            EDM_DVE = int(os.environ.get("EDM_DVE", "1"))  # groups on DVE (of 4)

            for img in range(IMG_PER_CORE):
                for s in range(N_STRIPS):
                    t0 = s * STRIP
                    if img == 0 and s == 0:
                        xa, xb = pre_x
                    else:
                        xa = xp.tile([128, STRIP], dt.bfloat16, tag="xa", bufs=6)
                        nc.sync.dma_start(out=xa[:, 0:256],
                                          in_=xT[img, 0:128, t0:t0 + 256])
                        nc.sync.dma_start(out=xa[:, 256:512],
                                          in_=xT[img, 0:128, t0 + 256:t0 + 512])
                        xb = xp.tile([128, STRIP], dt.bfloat16, tag="xb", bufs=6)
                        nc.sync.dma_start(out=xb[:, 0:256],
                                          in_=xT[img, 128:256, t0:t0 + 256])
                        nc.sync.dma_start(out=xb[:, 256:512],
                                          in_=xT[img, 128:256, t0 + 256:t0 + 512])

                    # ---- q/k projection: out [o=128 (4 heads), t=512]
                    # order q03, k03, q47, k47: head-group 0's dots need
                    # only the first two copies
                    qk_sb = [None] * 4
                    for ot in (0, 2, 1, 3):
                        qkps = psp.tile([128, STRIP], dt.float32, tag="qkps",
                                        bufs=QKPS_BUFS)
                        nc.tensor.matmul(qkps, wqk_sb[0][:, ot * 128:(ot + 1) * 128],
                                         xa, start=True, stop=False)
                        nc.tensor.matmul(qkps, wqk_sb[1][:, ot * 128:(ot + 1) * 128],
                                         xb, start=False, stop=True)
                        qk_t = qkp.tile([128, STRIP], dt.bfloat16, tag="qk_t", bufs=12)
                        nc.any.tensor_copy(qk_t, qkps)
                        qk_sb[ot] = qk_t

                    # ---- attention: strip-batched dots -> exp -> mask-mult
                    # row-groups split across 2-bank half tiles: concurrent
                    # row tiles must not share a PSUM bank
                    # mask-mult split into wp-pair halves with separate edm
                    # tiles: AV for wp 0-1 gates only on the h0-half muls,
                    # which are emitted (= prioritized) before all h1 halves
                    edm_sb = {}
                    ed_sb = {}
                    for hg in range(2):
                        for half in range(2):
                            dps = psp.tile([128, 2, 512], dt.float32,
                                           tag="dps", bufs=DPS_BUFS)
                            for wp in range(NWP):
                                c0 = wp * 128
                                for r2 in range(2):
                                    rg = 2 * half + r2
                                    nc.tensor.matmul(
                                        dps[:, r2, c0:c0 + 128],
                                        qk_sb[2 + hg][32 * rg:32 * rg + 32,
                                                      c0:c0 + 128],
                                        qk_sb[hg][32 * rg:32 * rg + 32,
                                                  c0:c0 + 128],
                                        start=True, stop=True,
                                        tile_position=(32 * rg, 0),
                                    )
                            ed = ep.tile([128, 2, 512], dt.bfloat16, tag="ed", bufs=10)
                            nc.scalar.activation(ed, dps, EXP)
                            ed_sb[(hg, half)] = ed
                    for wh in range(2):
                        cs = slice(wh * 256, wh * 256 + 256)
                        for hg in range(2):
                            for half in range(2):
                                edm = ep.tile([128, 2, 256], dt.bfloat16,
                                              tag="edm", bufs=24)
                                gi = 2 * hg + half
                                if gi >= 4 - EDM_DVE:
                                    nc.vector.tensor_mul(
                                        edm, ed_sb[(hg, half)][:, :, cs],
                                        eb_sb[:, 2 * half:2 * half + 2, cs])
                                else:
                                    nc.gpsimd.tensor_mul(
                                        edm, ed_sb[(hg, half)][:, :, cs],
                                        eb_sb[:, 2 * half:2 * half + 2, cs])
                                edm_sb[(hg, half, wh)] = edm

                    # ---- v projection: out [t=128, 8, 32] + ones col
                    # (emitted after dots so the dots->exp->mask critical
                    # path gets engine priority; v only gates AV)
                    v_sb = []
                    for tb in range(NWP):
                        vps = psp.tile([128, NHEADS, CH], dt.float32,
                                       tag="qkps", bufs=QKPS_BUFS)
                        nc.tensor.matmul(vps, xa[:, tb * 128:(tb + 1) * 128],
                                         wv_sb[0], start=True, stop=False)
                        nc.tensor.matmul(vps, xb[:, tb * 128:(tb + 1) * 128],
                                         wv_sb[1], start=False, stop=True)
                        v3 = vp.tile([128, NHEADS, CH + 1], dt.bfloat16,
                                     tag="v3", bufs=12)
                        nc.gpsimd.memset(v3[:, :, CH:CH + 1], 1.0)
                        nc.any.tensor_copy(v3[:, :, 0:CH], vps)
                        v_sb.append(v3)

                    fo2 = None
                    for wp in range(NWP):
                        c0 = wp * 128
                        avps = psp.tile([128, NHEADS, CH + 1], dt.float32,
                                        tag="smallps", bufs=SMALLPS_BUFS)
                        cc = (wp % 2) * 128
                        for hg in range(2):
                            for rg in range(4):
                                h = 4 * hg + rg
                                nc.tensor.matmul(
                                    avps[:, h, :],
                                    edm_sb[(hg, rg // 2, wp // 2)][:, rg % 2,
                                                                   cc:cc + 128],
                                    v_sb[wp][:, h, :],
                                    start=True, stop=True,
                                )
                        # normalize: attnout = av * (1/den)
                        rd = rdp.tile([128, NHEADS, 1], dt.float32, tag="rd", bufs=8)
                        nc.vector.reciprocal(rd, avps[:, :, CH:CH + 1])
                        attn = anp.tile([128, NHEADS, CH], dt.bfloat16,
                                        tag="attn", bufs=8)
                        nc.vector.tensor_mul(attn, avps[:, :, 0:CH],
                                             rd.to_broadcast((128, NHEADS, CH)))

                        # transpose [t,hc] -> [hc,t]
                        tps = psp.tile([128, 2, 128], dt.bfloat16,
                                       tag="smallps", bufs=SMALLPS_BUFS)
                        for half in range(2):
                            nc.tensor.transpose(
                                tps[:, half, :],
                                attn[:, half * 4:(half + 1) * 4, :], id_sb)
                        aot = aotp.tile([128, 2, 128], dt.bfloat16, tag="aot", bufs=8)
                        nc.any.tensor_copy(aot, tps)
                        aot_sb = [aot[:, 0, :], aot[:, 1, :]]

                        # out projection (b_out added host-side)
                        ops = psp.tile([128, 256], dt.float32, tag="smallps",
                                       bufs=SMALLPS_BUFS)
                        nc.tensor.matmul(ops, aot_sb[0], wout_sb[0],
                                         start=True, stop=False)
                        nc.tensor.matmul(ops, aot_sb[1], wout_sb[1],
                                         start=False, stop=True)
                        fo = fop.tile([128, 256], dt.bfloat16, tag="fo",
                                      bufs=6)
                        nc.any.tensor_copy(fo, ops)
                        nc.sync.dma_start(
                            out=out[img, t0 + c0:t0 + c0 + 128, :], in_=fo)

            # HAM keep-alive: dummy standalone LDWEIGHTS, emitted last so
            # they have the lowest scheduling priority and fill PE idle
            # gaps only.  No PSUM output, no consumers; every real matmul
            # self-loads its weights so a stale dummy load is harmless.
            # Keeps the PE activity monitor from re-throttling the clock
            # to 1.2 GHz during dependency stalls.
            for _ in range(int(os.environ.get("HAM_DUMMIES", "2000"))):
                nc.tensor.ldweights(id_sb)
    nc.compile()
    return nc


def _host_prep(x, w_qkv, w_out, b_out, pos_embedding):
    ws = WS
    scale = CH ** -0.5
    xs = x.reshape(B * L, H // ws, ws, W // ws, ws, C)
    xs = xs.transpose(0, 1, 3, 2, 4, 5).reshape(IMG, T_IMG, C)
    xT = np.ascontiguousarray(xs.transpose(0, 2, 1)).astype(BF16)

    wq = (w_qkv[:, 0:256] * scale).astype(BF16)
    wk = w_qkv[:, 256:512].astype(BF16)
    wqk = np.concatenate([wq, wk], axis=1)
    wv = w_qkv[:, 512:768].astype(BF16)

    ri = _relative_indices(ws)
    bias = pos_embedding[ri[:, :, 0], ri[:, :, 1]]  # [i, j]
    ebT = np.exp(bias.astype(np.float64)).T.astype(np.float32)  # [j, i]
    ebsuper = np.zeros((128, 128), np.float32)
    ebsuper[0:64, 0:64] = ebT
    ebsuper[64:128, 64:128] = ebT
    ebrep = np.tile(ebsuper, (1, 16)).astype(BF16)

    ident = np.eye(128, dtype=BF16)

    return {
        "xT": xT,
        "wqk": np.ascontiguousarray(wqk),
        "wv": np.ascontiguousarray(wv),
        "wout": w_out.astype(BF16),
        "ebrep": ebrep,
        "ident": ident,
    }


def kernel(x, w_qkv, w_out, b_out, pos_embedding, window_size, **extra):
    from concourse.bass_utils import run_bass_kernel_spmd

    x = np.asarray(x, dtype=np.float32)
    w_qkv = np.asarray(w_qkv, dtype=np.float32)
    w_out = np.asarray(w_out, dtype=np.float32)
    b_out = np.asarray(b_out, dtype=np.float32)
    pos_embedding = np.asarray(pos_embedding, dtype=np.float32)

    prep = _host_prep(x, w_qkv, w_out, b_out, pos_embedding)

    if "nc" not in _CACHE:
        _CACHE["nc"] = _build_kernel()
    nc = _CACHE["nc"]

    in_maps = []
    for core in range(N_CORES):
        m = dict(prep)
        m["xT"] = np.ascontiguousarray(
            prep["xT"][core * IMG_PER_CORE:(core + 1) * IMG_PER_CORE])
        in_maps.append(m)

    res = run_bass_kernel_spmd(nc, in_maps, core_ids=list(range(N_CORES)))
    outs = [res.results[c]["out"] for c in range(N_CORES)]
    o = np.concatenate(outs, axis=0)  # [16, 4096, 256]
    o = o.reshape(B * L, H // WS, W // WS, WS, WS, C)
    o = o.transpose(0, 1, 3, 2, 4, 5).reshape(B, L, H, W, C)
    o = o.astype(np.float32)
    o += b_out.astype(np.float32)
    return np.ascontiguousarray(o)



# revision 51
# speedup vs baseline: 1.4170x; 1.4170x over previous
"""Trainium2 Bass kernel for BaseWindowAttention.

Problem (hardcoded): x [2,8,64,64,256] f32, w_qkv [256,768], w_out [256,256],
b_out [256], pos_embedding [15,15], window_size 8, heads 8, dim_head 32.

Strategy:
- Data parallel: 16 (b,l) images over 8 cores -> 2 images/core.
- Host: window-major channel-first bf16 transpose of x; fold softmax scale
  into w_q; precompute exp(bias) 2-window super-tile (off-diagonal zeros kill
  cross-window attention terms); b_out added host-side after the gather.
- Device per core, per strip of 512 tokens (8 windows):
  q/k projection ([o,t] layout), v projection ([t,o] layout, head-strided with
  an appended ones column for the softmax denominator), window-pair dots as
  4 row-group-packed [32,128]x[32,128] matmuls, ACT exp, GpSimd/DVE multiply
  by exp(bias) mask tile, AV matmul (fused denominator), reciprocal +
  broadcast normalize, PE transpose to [hc,t], out-projection, per-wp
  output DMA.
- PSUM banks: qk/v projections share a 2-buf [128,512] tag, dots use two
  2-bank tiles, av/transpose/out-proj share a 2-buf small tag (8 banks
  total).  Split x DMAs ([128,256] quarters) shorten prefetch latency.
  Copies routed via nc.any so the Tile scheduler balances ACT/DVE; the
  exp(bias) mask multiply is split 3 GpSimd / 1 DVE.

Perf notes (measured on trn2 via NTFF): baseline 294us -> this version
~197us.  PE (tensor engine) is the critical resource: LDWEIGHTS fully
overlaps MATMUL, so instruction *time* not load count dominates; HAM
p-state (1.2 vs 2.4 GHz) oscillates with PE idle gaps, so PSUM slot
availability (qkps bufs=2) and deep SBUF pools matter more than per-op
engine choice.  The largest single win after PSUM rebalancing was
splitting each exp(bias) mask-multiply into two wp-pair halves with
separate edm tiles, h0 halves emitted before h1 halves (218 -> 197us):
AV for wp 0-1 then gates on only half the serialized GpSimd mul chain.
EDM_DVE=1 beats 2 or 4 in both structures.  Attempted and rejected:
DMA-XBAR transpose (Sync engine serializes ~1.2us/transpose -> 2x
slower), per-window garbage-free dots via column tile_position packing
(two stationary tiles sharing a PE row band corrupt results: one
window's outputs zero out), fp8 projections (error budget too tight).
"""

import os
import sys
import numpy as np

sys.path.insert(0, "/opt/trn_rl_repo")
os.environ.setdefault("JAX_PLATFORMS", "")

import ml_dtypes

BF16 = ml_dtypes.bfloat16

B, L, H, W, C = 2, 8, 64, 64, 256
WS = 8
NHEADS = 8
CH = 32
N_CORES = 8
IMG = B * L                 # 16 images
IMG_PER_CORE = IMG // N_CORES
T_IMG = H * W               # 4096 tokens per image
STRIP = 512                 # tokens per strip (8 windows)
N_STRIPS = T_IMG // STRIP   # 8
NWP = STRIP // 128          # 4 window pairs per strip

_CACHE = {}


def _relative_indices(ws):
    idx = np.array([[i, j] for i in range(ws) for j in range(ws)])
    rel = idx[None, :, :] - idx[:, None, :] + ws - 1
    return rel


def _build_kernel():
    import concourse.bass as bass
    import concourse.mybir as mybir
    import concourse.tile as tile
    from concourse import bacc

    dt = mybir.dt
    nc = bacc.Bacc("TRN2", target_bir_lowering=False, debug=False)

    xT = nc.dram_tensor("xT", [IMG_PER_CORE, C, T_IMG], dt.bfloat16,
                        kind="ExternalInput").ap()
    wqk = nc.dram_tensor("wqk", [C, 512], dt.bfloat16, kind="ExternalInput").ap()
    wv = nc.dram_tensor("wv", [C, C], dt.bfloat16, kind="ExternalInput").ap()
    wout = nc.dram_tensor("wout", [C, C], dt.bfloat16, kind="ExternalInput").ap()
    ebrep = nc.dram_tensor("ebrep", [128, 2048], dt.bfloat16,
                           kind="ExternalInput").ap()
    ident = nc.dram_tensor("ident", [128, 128], dt.bfloat16,
                           kind="ExternalInput").ap()
    out = nc.dram_tensor("out", [IMG_PER_CORE, T_IMG, C], dt.bfloat16,
                         kind="ExternalOutput").ap()

    EXP = mybir.ActivationFunctionType.Exp

    with tile.TileContext(nc) as tc:
        from contextlib import ExitStack
        with ExitStack() as ctx:
            consts = ctx.enter_context(tc.tile_pool(name="consts", bufs=1))
            xp = ctx.enter_context(tc.tile_pool(name="xp", bufs=3))
            qkp = ctx.enter_context(tc.tile_pool(name="qkp", bufs=8))
            vp = ctx.enter_context(tc.tile_pool(name="vp", bufs=8))
            ep = ctx.enter_context(tc.tile_pool(name="ep", bufs=3))
            anp = ctx.enter_context(tc.tile_pool(name="anp", bufs=3))
            aotp = ctx.enter_context(tc.tile_pool(name="aotp", bufs=4))
            rdp = ctx.enter_context(tc.tile_pool(name="rdp", bufs=3))
            fop = ctx.enter_context(tc.tile_pool(name="fop", bufs=3))
            psp = ctx.enter_context(tc.tile_pool(name="psp", bufs=1, space="PSUM"))

            # ---- strip-0 inputs first (head latency), then constants.
            # Big constants are split so no single DMA engine serializes
            # a large transfer in front of the first matmuls.
            pre_x = []
            for half in range(2):
                xt = xp.tile([128, STRIP], dt.bfloat16,
                             tag=("xa" if half == 0 else "xb"), bufs=6)
                for q in range(2):
                    nc.sync.dma_start(
                        out=xt[:, q * 256:(q + 1) * 256],
                        in_=xT[0, half * 128:half * 128 + 128,
                               q * 256:(q + 1) * 256])
                pre_x.append(xt)

            wqk_sb = []
            wv_sb = []
            wout_sb = []
            for kk in range(2):
                wqk_t = consts.tile([128, 512], dt.bfloat16, tag=f"wqk{kk}")
                for q in range(2):
                    nc.sync.dma_start(
                        out=wqk_t[:, q * 256:(q + 1) * 256],
                        in_=wqk[kk * 128:(kk + 1) * 128,
                                q * 256:(q + 1) * 256])
                wqk_sb.append(wqk_t)
                wv_t = consts.tile([128, 256], dt.bfloat16, tag=f"wv{kk}")
                nc.sync.dma_start(out=wv_t, in_=wv[kk * 128:(kk + 1) * 128, :])
                wv_sb.append(wv_t)
                wout_t = consts.tile([128, 256], dt.bfloat16, tag=f"wout{kk}")
                nc.sync.dma_start(out=wout_t, in_=wout[kk * 128:(kk + 1) * 128, :])
                wout_sb.append(wout_t)
            eb_sb = consts.tile([128, 4, 512], dt.bfloat16, tag="eb")
            ebr = ebrep.rearrange("p (r c) -> p r c", r=4)
            for r in range(4):
                nc.sync.dma_start(out=eb_sb[:, r, :], in_=ebr[:, r, :])
            id_sb = consts.tile([128, 128], dt.bfloat16, tag="id")
            nc.sync.dma_start(out=id_sb, in_=ident)

            QKPS_BUFS = int(os.environ.get("QKPS_BUFS", "2"))
            SMALLPS_BUFS = int(os.environ.get("SMALLPS_BUFS", "2"))
            DPS_BUFS = int(os.environ.get("DPS_BUFS", "2"))
            EDM_DVE = int(os.environ.get("EDM_DVE", "1"))  # groups on DVE (of 4)

            for img in range(IMG_PER_CORE):
                for s in range(N_STRIPS):
                    t0 = s * STRIP
                    if img == 0 and s == 0:
                        xa, xb = pre_x
                    else:
                        xa = xp.tile([128, STRIP], dt.bfloat16, tag="xa", bufs=6)
                        nc.sync.dma_start(out=xa[:, 0:256],
                                          in_=xT[img, 0:128, t0:t0 + 256])
                        nc.sync.dma_start(out=xa[:, 256:512],
                                          in_=xT[img, 0:128, t0 + 256:t0 + 512])
                        xb = xp.tile([128, STRIP], dt.bfloat16, tag="xb", bufs=6)
                        nc.sync.dma_start(out=xb[:, 0:256],
                                          in_=xT[img, 128:256, t0:t0 + 256])
                        nc.sync.dma_start(out=xb[:, 256:512],
                                          in_=xT[img, 128:256, t0 + 256:t0 + 512])

                    # ---- q/k projection: out [o=128 (4 heads), t=512]
                    # order q03, k03, q47, k47: head-group 0's dots need
                    # only the first two copies
                    qk_sb = [None] * 4
                    for ot in (0, 2, 1, 3):
                        qkps = psp.tile([128, STRIP], dt.float32, tag="qkps",
                                        bufs=QKPS_BUFS)
                        nc.tensor.matmul(qkps, wqk_sb[0][:, ot * 128:(ot + 1) * 128],
                                         xa, start=True, stop=False)
                        nc.tensor.matmul(qkps, wqk_sb[1][:, ot * 128:(ot + 1) * 128],
                                         xb, start=False, stop=True)
                        qk_t = qkp.tile([128, STRIP], dt.bfloat16, tag="qk_t", bufs=12)
                        nc.any.tensor_copy(qk_t, qkps)
                        qk_sb[ot] = qk_t

                    # ---- attention: strip-batched dots -> exp -> mask-mult
                    # row-groups split across 2-bank half tiles: concurrent
                    # row tiles must not share a PSUM bank
                    # mask-mult split into wp-pair halves with separate edm
                    # tiles: AV for wp 0-1 gates only on the h0-half muls,
                    # which are emitted (= prioritized) before all h1 halves
                    edm_sb = {}
                    ed_sb = {}
                    for hg in range(2):
                        for half in range(2):
                            dps = psp.tile([128, 2, 512], dt.float32,
                                           tag="dps", bufs=DPS_BUFS)
                            for wp in range(NWP):
                                c0 = wp * 128
                                for r2 in range(2):
                                    rg = 2 * half + r2
                                    nc.tensor.matmul(
                                        dps[:, r2, c0:c0 + 128],
                                        qk_sb[2 + hg][32 * rg:32 * rg + 32,
                                                      c0:c0 + 128],
                                        qk_sb[hg][32 * rg:32 * rg + 32,
                                                  c0:c0 + 128],
                                        start=True, stop=True,
                                        tile_position=(32 * rg, 0),
                                    )
                            ed = ep.tile([128, 2, 512], dt.bfloat16, tag="ed", bufs=10)
                            nc.scalar.activation(ed, dps, EXP)
                            ed_sb[(hg, half)] = ed
                    for wh in range(2):
                        cs = slice(wh * 256, wh * 256 + 256)
                        for hg in range(2):
                            for half in range(2):
                                edm = ep.tile([128, 2, 256], dt.bfloat16,
                                              tag="edm", bufs=24)
                                gi = 2 * hg + half
                                if gi >= 4 - EDM_DVE:
                                    nc.vector.tensor_mul(
                                        edm, ed_sb[(hg, half)][:, :, cs],
                                        eb_sb[:, 2 * half:2 * half + 2, cs])
                                else:
                                    nc.gpsimd.tensor_mul(
                                        edm, ed_sb[(hg, half)][:, :, cs],
                                        eb_sb[:, 2 * half:2 * half + 2, cs])
                                edm_sb[(hg, half, wh)] = edm

                    # ---- v projection: out [t=128, 8, 32] + ones col
                    # (emitted after dots so the dots->exp->mask critical
                    # path gets engine priority; v only gates AV)
                    v_sb = []
                    for tb in range(NWP):
                        vps = psp.tile([128, NHEADS, CH], dt.float32,
                                       tag="qkps", bufs=QKPS_BUFS)
                        nc.tensor.matmul(vps, xa[:, tb * 128:(tb + 1) * 128],
                                         wv_sb[0], start=True, stop=False)
                        nc.tensor.matmul(vps, xb[:, tb * 128:(tb + 1) * 128],
                                         wv_sb[1], start=False, stop=True)
                        v3 = vp.tile([128, NHEADS, CH + 1], dt.bfloat16,
                                     tag="v3", bufs=12)
                        nc.gpsimd.memset(v3[:, :, CH:CH + 1], 1.0)
                        nc.any.tensor_copy(v3[:, :, 0:CH], vps)
                        v_sb.append(v3)

                    fo2 = None
                    for wp in range(NWP):
                        c0 = wp * 128
                        avps = psp.tile([128, NHEADS, CH + 1], dt.float32,
                                        tag="smallps", bufs=SMALLPS_BUFS)
                        cc = (wp % 2) * 128
                        for hg in range(2):
                            for rg in range(4):
                                h = 4 * hg + rg
                                nc.tensor.matmul(
                                    avps[:, h, :],
                                    edm_sb[(hg, rg // 2, wp // 2)][:, rg % 2,
                                                                   cc:cc + 128],
                                    v_sb[wp][:, h, :],
                                    start=True, stop=True,
                                )
                        # normalize: attnout = av * (1/den)
                        rd = rdp.tile([128, NHEADS, 1], dt.float32, tag="rd", bufs=8)
                        nc.vector.reciprocal(rd, avps[:, :, CH:CH + 1])
                        attn = anp.tile([128, NHEADS, CH], dt.bfloat16,
                                        tag="attn", bufs=8)
                        nc.vector.tensor_mul(attn, avps[:, :, 0:CH],
                                             rd.to_broadcast((128, NHEADS, CH)))

                        # transpose [t,hc] -> [hc,t]
                        tps = psp.tile([128, 2, 128], dt.bfloat16,
                                       tag="smallps", bufs=SMALLPS_BUFS)
                        for half in range(2):
                            nc.tensor.transpose(
                                tps[:, half, :],
                                attn[:, half * 4:(half + 1) * 4, :], id_sb)
                        aot = aotp.tile([128, 2, 128], dt.bfloat16, tag="aot", bufs=8)
                        nc.any.tensor_copy(aot, tps)
                        aot_sb = [aot[:, 0, :], aot[:, 1, :]]

                        # out projection (b_out added host-side)
                        ops = psp.tile([128, 256], dt.float32, tag="smallps",
                                       bufs=SMALLPS_BUFS)
                        nc.tensor.matmul(ops, aot_sb[0], wout_sb[0],
                                         start=True, stop=False)
                        nc.tensor.matmul(ops, aot_sb[1], wout_sb[1],
                                         start=False, stop=True)
                        fo = fop.tile([128, 256], dt.bfloat16, tag="fo",
                                      bufs=6)
                        nc.any.tensor_copy(fo, ops)
                        nc.sync.dma_start(
                            out=out[img, t0 + c0:t0 + c0 + 128, :], in_=fo)

            # HAM pre-warm: dummy standalone LDWEIGHTS sized to fit inside
            # the ~10us DMA-bound head (the only PE-idle window where the
            # scheduler can place them, since nothing else is ready there).
            # ~8us of PE activity un-throttles the clock gate before the
            # first real matmul.  2000 of these regressed +93us (they all
            # front-load); the count must stay under the head idle.
            for _ in range(int(os.environ.get("HAM_DUMMIES", "170"))):
                nc.tensor.ldweights(id_sb)
    nc.compile()
    return nc


def _host_prep(x, w_qkv, w_out, b_out, pos_embedding):
    ws = WS
    scale = CH ** -0.5
    xs = x.reshape(B * L, H // ws, ws, W // ws, ws, C)
    xs = xs.transpose(0, 1, 3, 2, 4, 5).reshape(IMG, T_IMG, C)
    xT = np.ascontiguousarray(xs.transpose(0, 2, 1)).astype(BF16)

    wq = (w_qkv[:, 0:256] * scale).astype(BF16)
    wk = w_qkv[:, 256:512].astype(BF16)
    wqk = np.concatenate([wq, wk], axis=1)
    wv = w_qkv[:, 512:768].astype(BF16)

    ri = _relative_indices(ws)
    bias = pos_embedding[ri[:, :, 0], ri[:, :, 1]]  # [i, j]
    ebT = np.exp(bias.astype(np.float64)).T.astype(np.float32)  # [j, i]
    ebsuper = np.zeros((128, 128), np.float32)
    ebsuper[0:64, 0:64] = ebT
    ebsuper[64:128, 64:128] = ebT
    ebrep = np.tile(ebsuper, (1, 16)).astype(BF16)

    ident = np.eye(128, dtype=BF16)

    return {
        "xT": xT,
        "wqk": np.ascontiguousarray(wqk),
        "wv": np.ascontiguousarray(wv),
        "wout": w_out.astype(BF16),
        "ebrep": ebrep,
        "ident": ident,
    }


def kernel(x, w_qkv, w_out, b_out, pos_embedding, window_size, **extra):
    from concourse.bass_utils import run_bass_kernel_spmd

    x = np.asarray(x, dtype=np.float32)
    w_qkv = np.asarray(w_qkv, dtype=np.float32)
    w_out = np.asarray(w_out, dtype=np.float32)
    b_out = np.asarray(b_out, dtype=np.float32)
    pos_embedding = np.asarray(pos_embedding, dtype=np.float32)

    prep = _host_prep(x, w_qkv, w_out, b_out, pos_embedding)

    if "nc" not in _CACHE:
        _CACHE["nc"] = _build_kernel()
    nc = _CACHE["nc"]

    in_maps = []
    for core in range(N_CORES):
        m = dict(prep)
        m["xT"] = np.ascontiguousarray(
            prep["xT"][core * IMG_PER_CORE:(core + 1) * IMG_PER_CORE])
        in_maps.append(m)

    res = run_bass_kernel_spmd(nc, in_maps, core_ids=list(range(N_CORES)))
    outs = [res.results[c]["out"] for c in range(N_CORES)]
    o = np.concatenate(outs, axis=0)  # [16, 4096, 256]
    o = o.reshape(B * L, H // WS, W // WS, WS, WS, C)
    o = o.transpose(0, 1, 3, 2, 4, 5).reshape(B, L, H, W, C)
    o = o.astype(np.float32)
    o += b_out.astype(np.float32)
    return np.ascontiguousarray(o)

